# revision 17
# baseline (speedup 1.0000x reference)
"""MLA (Multi-head Latent Attention) Bass/Tile kernel for 8 Trainium2 NeuronCores.

Problem: nn_MultiHeadLatentAttention_81707457839331
  B=2, S=2048, HID=2048, NH=16 heads, NOPE=128, ROPE=64, VD=128, QKD=192,
  KVR=512, QR=1536, fp32 in/out.

Sharding (single NEFF, SPMD on 8 cores):
  core c -> batch b = c//4, head group g = c%4 (4 heads each).
  The shared down-projection is SEQUENCE-PARALLEL inside each 4-core batch
  group: core g computes its 512-token slice of all 2304 latent features,
  then two AllGathers (kv chunks first, q-latent chunks second) replicate
  the full latents to the group. q_up/kv_up/attention/o_proj are
  head-sharded. Each core emits a partial o_proj output [S, HID]; the host
  sums the 4 partials per batch.

Perf notes vs the fp32r baseline (967us):
  - all matmul inputs are bf16 (same PE rate as f32r at moving dim 512, but
    1.0 cyc/row LDWEIGHTS instead of 1.5, half the DMA/SBUF footprint;
    numpy-sim puts the end-to-end rel err at ~5.5e-3 vs the 2e-2 gate),
  - loop orders reuse each stationary across all 4 token tiles, cutting
    LDWEIGHTS count ~3x (the baseline spent 484us in LDW),
  - RMSNorm partition reductions + softmax denominators run on the PE as
    ones-vector matmuls accumulated in PSUM (the baseline burned ~90us of
    GpSimd partition_all_reduce + 80us of DVE full-tile reciprocals),
  - per-token scales are broadcast to 128 partitions with a rank-1 PE
    matmul instead of gpsimd,
  - rotate_half for q-rope is a PE permutation matmul (host-built +-1
    matrix), dropping the 2 extra rot weight chunks from q_up,
  - causal mask is applied post-exp as a bf16 0/1 multiply on DVE,
  - kt/kre/kro/v/qT/attention-out all stay in SBUF between phases.
"""

import numpy as np
import ml_dtypes

import concourse.bass as bass
import concourse.bass_isa as bass_isa
import concourse.mybir as mybir
import concourse.tile as tile
from concourse import bacc
from concourse.bass import ds, ts
from concourse.bass_utils import run_bass_kernel_spmd

F32 = mybir.dt.float32
F32R = mybir.dt.float32r
BF16 = mybir.dt.bfloat16
AF = mybir.ActivationFunctionType
NPBF = ml_dtypes.bfloat16

B, S, HID, NH = 2, 2048, 2048, 16
NOPE, ROPE, VD = 128, 64, 128
QKD = NOPE + ROPE
KVR, QR = 512, 1536
EPS = 1e-6
SCALE = QKD ** (-0.5)
P = 128

NHC = HID // P            # 16 hidden chunks
NQC = QR // P             # 12 q-latent chunks
NKC = KVR // P            # 4 ckv chunks
NFC = 6 + NQC             # down-proj chunks: 4 ckv + rope-dup + rot-dup + 12 q
NDQ = 6                   # q_up output chunks: 4 nope + 2 rope-pairs
NTT = S // 512            # 4 token tiles of 512
NTC = S // P              # 16 token chunks of 128
RG = [[0, 1, 2, 3], [4, 5, 6, 7]]
DEBUG = False


def _emit(tc):
    nc = tc.nc
    hid_in = nc.dram_tensor("hid", [P, NHC, 512], BF16, kind="ExternalInput").ap()
    cos_in = nc.dram_tensor("cos2", [P, S], F32, kind="ExternalInput").ap()
    sin_in = nc.dram_tensor("sin2", [P, S], F32, kind="ExternalInput").ap()
    wd_in = nc.dram_tensor("wd", [NFC, P, NHC, P], BF16, kind="ExternalInput").ap()
    wqup_in = nc.dram_tensor("wqup", [P, NQC, NDQ * P], BF16, kind="ExternalInput").ap()
    wkup_in = nc.dram_tensor("wkup", [P, NKC, 512], BF16, kind="ExternalInput").ap()
    wvup_in = nc.dram_tensor("wvup", [P, NKC, 512], BF16, kind="ExternalInput").ap()
    wo_in = nc.dram_tensor("wo", [P, 4, HID], BF16, kind="ExternalInput").ap()
    prot_in = nc.dram_tensor("prot", [P, P], BF16, kind="ExternalInput").ap()
    out_d = nc.dram_tensor("out", [S, HID], BF16, kind="ExternalOutput").ap()
    if DEBUG:
        dbg_kv = nc.dram_tensor("dbg_kv", [P, 6, NTT, 512], BF16,
                                kind="ExternalOutput").ap()
        dbg_kt = nc.dram_tensor("dbg_kt", [P, 4, S], BF16,
                                kind="ExternalOutput").ap()
        dbg_kre = nc.dram_tensor("dbg_kre", [P, S], BF16,
                                 kind="ExternalOutput").ap()
        dbg_kro = nc.dram_tensor("dbg_kro", [P, S], BF16,
                                 kind="ExternalOutput").ap()
        dbg_v = nc.dram_tensor("dbg_v", [P, NTC, 512], BF16,
                               kind="ExternalOutput").ap()
        dbg_qt = nc.dram_tensor("dbg_qt", [P, 6, S], BF16,
                                kind="ExternalOutput").ap()
        dbg_o = nc.dram_tensor("dbg_o", [P, 4, S], BF16,
                               kind="ExternalOutput").ap()
        dbg_mask = nc.dram_tensor("dbg_mask", [P, 4, 512], BF16,
                                  kind="ExternalOutput").ap()

    with (
        tc.tile_pool(name="const", bufs=1) as constp,
        tc.tile_pool(name="dram", bufs=1, space="DRAM") as dramp,
    ):
        eps_kv = constp.tile([1, 1], F32)
        nc.vector.memset(eps_kv, EPS)
        eps_q = constp.tile([1, 1], F32)
        nc.vector.memset(eps_q, EPS / (SCALE * SCALE))
        ones_f32 = constp.tile([P, 1], F32)
        nc.vector.memset(ones_f32, 1.0)
        ones_kf = constp.tile([P, 1], F32R)       # colsum stationary
        nc.vector.tensor_copy(ones_kf, ones_f32)
        ones_b = constp.tile([1, P], F32R)        # rank-1 row broadcast
        ones_bf32 = constp.tile([1, P], F32)
        nc.vector.memset(ones_bf32, 1.0)
        nc.vector.tensor_copy(ones_b, ones_bf32)
        # causal 0/1 mask for the (narrowed) diagonal: mask[p,x] = x >= p
        mask0 = constp.tile([P, 512], BF16, name="mask0")
        nc.gpsimd.memset(mask0, 1.0)
        nc.gpsimd.affine_select(
            out=mask0, in_=mask0, pattern=[[1, 512]],
            compare_op=mybir.AluOpType.is_ge, fill=0.0,
            base=0, channel_multiplier=-1,
        )
        if DEBUG:
            for k in range(4):
                nc.sync.dma_start(dbg_mask[:, k, :], mask0)

        # AllGather bounce buffers (token-slice in, full-sequence out)
        ag_in = dramp.tile([NFC + 2, P, 512], BF16)
        ag_kv = dramp.tile([NTT, 7, P, 512], BF16)
        ag_q = dramp.tile([NTT, NQC + 1, P, 512], BF16)

        # ---------------- Phase A: seq-parallel down-projection ----------
        with (
            tc.tile_pool(name="pa", bufs=1) as pa,
            tc.tile_pool(name="pa_row", bufs=3) as par,
            tc.tile_pool(name="pa_ps", bufs=3, space="PSUM") as pap,
            tc.tile_pool(name="pa_st", bufs=2, space="PSUM") as pbst,
        ):
            with nc.named_scope("phaseA"):
                hid_sb = pa.tile([P, NHC, 512], BF16)
                nc.sync.dma_start(hid_sb[:, 0, :], hid_in[:, 0, :])
                wd_sb = pa.tile([P, NFC, NHC, P], BF16)
                nc.sync.dma_start(wd_sb[:, 0], wd_in[0])
                for hc in range(1, NHC):
                    nc.sync.dma_start(hid_sb[:, hc, :], hid_in[:, hc, :])
                for fc in range(1, NFC):
                    nc.sync.dma_start(wd_sb[:, fc], wd_in[fc])
                # producer-side RMSNorm square-sums, shipped in the gathers
                sacc_kv = pa.tile([P, 512], F32R)
                sacc_q = pa.tile([P, 512], F32R)
                for fc in range(NFC):
                    ps = pap.tile([P, 512], F32, name="aps")
                    for hc in range(NHC):
                        nc.tensor.matmul(
                            ps, wd_sb[:, fc, hc, :], hid_sb[:, hc, :],
                            start=(hc == 0), stop=(hc == NHC - 1),
                        )
                    row = par.tile([P, 512], BF16, name="arow")
                    nc.vector.tensor_copy(row, ps)
                    nc.sync.dma_start(ag_in[fc if fc < 6 else fc + 1], row)
                    if fc < 4 or fc >= 6:
                        sacc = sacc_kv if fc < 4 else sacc_q
                        first = fc in (0, 6)
                        if first:
                            nc.scalar.square(sacc, row)
                        else:
                            sq = par.tile([P, 512], F32, name="asq")
                            nc.scalar.square(sq, row)
                            nc.gpsimd.tensor_add(sacc, sacc, sq)
                    if fc == 3:
                        ssp = pbst.tile([1, 512], F32, name="bst")
                        nc.tensor.matmul(ssp, ones_kf, sacc_kv,
                                         start=True, stop=True)
                        ssr = par.tile([1, 512], BF16, name="assr")
                        nc.vector.tensor_copy(ssr, ssp)
                        nc.sync.dma_start(ag_in[6][0:1, :], ssr)
                    if fc == 5:
                        nc.gpsimd.collective_compute(
                            "AllGather", mybir.AluOpType.bypass, RG,
                            ins=[ag_in[0:7].opt()], outs=[ag_kv[:].opt()],
                        )
                ssp = pbst.tile([1, 512], F32, name="bst")
                nc.tensor.matmul(ssp, ones_kf, sacc_q, start=True, stop=True)
                ssr = par.tile([1, 512], BF16, name="assr")
                nc.vector.tensor_copy(ssr, ssp)
                nc.sync.dma_start(ag_in[NFC + 1][0:1, :], ssr)
                nc.gpsimd.collective_compute(
                    "AllGather", mybir.AluOpType.bypass, RG,
                    ins=[ag_in[7:NFC + 2].opt()], outs=[ag_q[:].opt()],
                )

        # kt/kre/kro/v stay in SBUF from phase B through phase D
        with tc.tile_pool(name="pkv", bufs=1) as pkv:
            kt_sb = pkv.tile([P, 4, S], BF16)     # 4 heads k_nope.T
            kre_sb = pkv.tile([P, S], BF16)       # [k_roped; 0]
            kro_sb = pkv.tile([P, S], BF16)       # [0; k_roped]
            v_sb = pkv.tile([P, NTC, 512], BF16)  # V in [token, 4*VD]
            cos_sb = pkv.tile([P, S], F32)
            sin_sb = pkv.tile([P, S], F32)
            nc.sync.dma_start(cos_sb, cos_in)
            nc.sync.dma_start(sin_sb, sin_in)
            nc.vector.memset(kre_sb[64:128, :], 0.0)
            nc.vector.memset(kro_sb[0:64, :], 0.0)

            # ---------------- Phase B: kv norm + rope-k + kv_up ----------
            with (
                tc.tile_pool(name="pb", bufs=1) as pb,
                tc.tile_pool(name="pb_t", bufs=3) as pbt,
                tc.tile_pool(name="pb_bc", bufs=1, space="PSUM") as pbbc,
                tc.tile_pool(name="pb_ps", bufs=4, space="PSUM") as pbps,
            ):
                with nc.named_scope("phaseB"):
                    kv_sb = pb.tile([P, 6, NTT, 512], BF16)
                    for tt in range(NTT):
                        for fc in range(6):
                            nc.sync.dma_start(kv_sb[:, fc, tt, :],
                                              ag_kv[tt, fc])
                    if DEBUG:
                        nc.sync.dma_start(dbg_kv, kv_sb)
                    kvn = pb.tile([P, NKC, NTT, 512], BF16)
                    sskv = pb.tile([1, NTT, 512], BF16)
                    for tt in range(NTT):
                        nc.sync.dma_start(sskv[:, tt, :], ag_kv[tt, 6][0:1, :])
                    for tt in range(NTT):
                        eng = nc.vector if tt % 2 == 0 else nc.gpsimd
                        srt = pbt.tile([1, 512], F32, name="bsrt")
                        nc.scalar.activation(srt, sskv[:, tt, :], AF.Ln,
                                             bias=eps_kv, scale=1.0 / KVR)
                        rk = pbt.tile([1, 512], F32R, name="brk")
                        nc.scalar.activation(rk, srt, AF.Exp, scale=-0.5)
                        bc = pbbc.tile([P, 512], F32, name="bbc")
                        nc.tensor.matmul(bc, ones_b, rk, start=True, stop=True)
                        bcs = pbt.tile([P, 512], F32, name="bbcs")
                        nc.vector.tensor_copy(bcs, bc)
                        for fc in range(NKC):
                            eng.tensor_mul(kvn[:, fc, tt, :],
                                           kv_sb[:, fc, tt, :], bcs)
                        # shared rope key (chunk 4 = [kr|kr], 5 = [rot|rot])
                        t1 = pbt.tile([P, 512], F32, name="bt1")
                        nc.vector.tensor_mul(t1, kv_sb[:, 4, tt, :],
                                             cos_sb[:, ts(tt, 512)])
                        t2 = pbt.tile([P, 512], F32, name="bt2")
                        nc.vector.tensor_mul(t2, kv_sb[:, 5, tt, :],
                                             sin_sb[:, ts(tt, 512)])
                        nc.vector.tensor_add(t1, t1, t2)
                        nc.vector.tensor_copy(kre_sb[0:64, ts(tt, 512)],
                                              t1[0:64, :])
                        nc.vector.tensor_copy(kro_sb[64:128, ts(tt, 512)],
                                              t1[64:128, :])
                    # kv_up: k-heads (stationary reused over token tiles)
                    wk_sb = pb.tile([P, NKC, 512], BF16)
                    nc.sync.dma_start(wk_sb, wkup_in)
                    wv_sb = pb.tile([P, NKC, 512], BF16)
                    nc.sync.dma_start(wv_sb, wvup_in)
                    for d in range(4):
                        pss = [pbps.tile([P, 512], F32, name="bkp")
                               for _ in range(NTT)]
                        for fc in range(NKC):
                            for tt in range(NTT):
                                nc.tensor.matmul(
                                    pss[tt], wk_sb[:, fc, ds(d * P, P)],
                                    kvn[:, fc, tt, :],
                                    start=(fc == 0), stop=(fc == NKC - 1),
                                )
                        for tt in range(NTT):
                            nc.vector.tensor_copy(kt_sb[:, d, ts(tt, 512)],
                                                  pss[tt])
                    # V in [token, head*vd]
                    for tch in range(NTC):
                        ps = pbps.tile([P, 512], F32, name="bkp")
                        for fc in range(NKC):
                            nc.tensor.matmul(
                                ps, kvn[:, fc, tch // 4, ds((tch % 4) * P, P)],
                                wv_sb[:, fc, :],
                                start=(fc == 0), stop=(fc == NKC - 1),
                            )
                        nc.vector.tensor_copy(v_sb[:, tch, :], ps)

            if DEBUG:
                nc.sync.dma_start(dbg_kt, kt_sb)
                nc.sync.dma_start(dbg_kre, kre_sb)
                nc.sync.dma_start(dbg_kro, kro_sb)
                nc.sync.dma_start(dbg_v, v_sb)

            # qT lives in SBUF from phase C through phase D
            with tc.tile_pool(name="pq", bufs=1) as pq:
                qT = pq.tile([P, 6, S], BF16)  # 4 nope + 2 roped pairs

                # ------------- Phase C: q_up + rope-q + q-norm -----------
                with (
                    tc.tile_pool(name="pc", bufs=1) as pc,
                    tc.tile_pool(name="pc_t", bufs=2) as pct,
                    tc.tile_pool(name="pc_st", bufs=2, space="PSUM") as pcst,
                    tc.tile_pool(name="pc_bc", bufs=2, space="PSUM") as pcbc,
                    tc.tile_pool(name="pc_ps", bufs=4, space="PSUM") as pcps,
                ):
                    with nc.named_scope("phaseC"):
                        wq_sb = pc.tile([P, NQC, NDQ * P], BF16)
                        nc.sync.dma_start(wq_sb, wqup_in)
                        prot_sb = pc.tile([P, P], BF16)
                        nc.sync.dma_start(prot_sb, prot_in)
                        latq = pc.tile([P, NQC, NTT, 512], BF16)
                        for tt in range(NTT):
                            for fc in range(NQC):
                                nc.sync.dma_start(latq[:, fc, tt, :],
                                                  ag_q[tt, fc])
                        ssq = pc.tile([1, NTT, 512], BF16)
                        for tt in range(NTT):
                            nc.sync.dma_start(ssq[:, tt, :],
                                              ag_q[tt, NQC][0:1, :])
                        rqs = []
                        for tt in range(NTT):
                            srt = pct.tile([1, 512], F32, name="csrt")
                            nc.scalar.activation(srt, ssq[:, tt, :], AF.Ln,
                                                 bias=eps_q,
                                                 scale=1.0 / (QR * SCALE * SCALE))
                            rk = pct.tile([1, 512], F32R, name="crk")
                            nc.scalar.activation(rk, srt, AF.Exp, scale=-0.5)
                            bc = pcbc.tile([P, 512], F32, name="cbc")
                            nc.tensor.matmul(bc, ones_b, rk,
                                             start=True, stop=True)
                            rq = pc.tile([P, 512], F32, name=f"crq{tt}")
                            nc.vector.tensor_copy(rq, bc)
                            rqs.append(rq)
                        for d in range(NDQ):
                            pss = [pcps.tile([P, 512], F32, name="cqp")
                                   for _ in range(NTT)]
                            for fc in range(NQC):
                                for tt in range(NTT):
                                    nc.tensor.matmul(
                                        pss[tt], wq_sb[:, fc, ds(d * P, P)],
                                        latq[:, fc, tt, :],
                                        start=(fc == 0), stop=(fc == NQC - 1),
                                    )
                            if d < 4:
                                for tt in range(NTT):
                                    nc.vector.tensor_mul(
                                        qT[:, d, ts(tt, 512)], pss[tt],
                                        rqs[tt])
                            else:
                                # roped pair: rotate_half via PE permutation
                                for tt in range(NTT):
                                    qr = pct.tile([P, 512], BF16, name="cqr")
                                    nc.vector.tensor_copy(qr, pss[tt])
                                    rt = pcbc.tile([P, 512], F32, name="cbc")
                                    nc.tensor.matmul(rt, prot_sb, qr,
                                                     start=True, stop=True)
                                    t1 = pct.tile([P, 512], F32, name="ct1")
                                    nc.vector.tensor_mul(
                                        t1, qr, cos_sb[:, ts(tt, 512)])
                                    t2 = pct.tile([P, 512], F32, name="ct2")
                                    nc.vector.tensor_mul(
                                        t2, rt, sin_sb[:, ts(tt, 512)])
                                    nc.vector.tensor_add(t1, t1, t2)
                                    nc.vector.tensor_mul(
                                        qT[:, d, ts(tt, 512)], t1, rqs[tt])

                if DEBUG:
                    nc.sync.dma_start(dbg_qt, qT)
                # ---------------- Phase D: attention ---------------------
                with tc.tile_pool(name="po", bufs=1) as po:
                    o_sb = po.tile([P, 4, S], BF16)
                    with (
                        tc.tile_pool(name="pd_e", bufs=6) as pde,
                        tc.tile_pool(name="pd_a", bufs=8) as pda,
                        tc.tile_pool(name="pd_t", bufs=4) as pdt,
                        tc.tile_pool(name="pd_sc", bufs=3, space="PSUM") as pdsc,
                        tc.tile_pool(name="pd_o", bufs=4, space="PSUM") as pdo,
                        tc.tile_pool(name="pd_den", bufs=1, space="PSUM") as pdd,
                    ):
                        with nc.named_scope("phaseD"):
                            for h in range(4):
                                krop = kre_sb if h % 2 == 0 else kro_sb
                                acc_eng = nc.vector if h % 2 == 0 else nc.gpsimd
                                qp = qT[:, 4 + h // 2, :]
                                ps_o = [pdo.tile([P, 512], F32, name="pso")
                                        for i in range(4)]
                                eaccs = [pda.tile([P, 512], F32R, name="eacc")
                                         for i in range(4)]
                                for jc in range(NTC):
                                    imin = jc // 4
                                    ets = {}
                                    for i in range(imin, 4):
                                        qoff = (jc % 4) * P if i == imin else 0
                                        w = 512 - qoff
                                        ps_sc = pdsc.tile([P, 512], F32,
                                                          name="psc")
                                        nc.tensor.matmul(
                                            ps_sc[:, :w],
                                            kt_sb[:, h, ds(jc * P, P)],
                                            qT[:, h, ds(i * 512 + qoff, w)],
                                            start=True, stop=False)
                                        nc.tensor.matmul(
                                            ps_sc[:, :w],
                                            krop[:, ds(jc * P, P)],
                                            qp[:, ds(i * 512 + qoff, w)],
                                            start=False, stop=True)
                                        et = pde.tile([P, 512], BF16,
                                                      name="et")
                                        nc.scalar.activation(et[:, :w],
                                                             ps_sc[:, :w],
                                                             AF.Exp)
                                        if i == imin:
                                            # beyond col 128 the narrowed
                                            # diag block is fully valid
                                            nc.vector.tensor_mul(
                                                et[:, :P], et[:, :P],
                                                mask0[:, :P])
                                        ets[i] = (et, qoff, w)
                                    for i in range(imin, 4):
                                        et, qoff, w = ets[i]
                                        nc.tensor.matmul(
                                            ps_o[i][:, ds(qoff, w)],
                                            v_sb[:, jc, ds(h * P, P)],
                                            et[:, :w],
                                            start=(jc == 0),
                                            stop=(jc == 4 * i + 3))
                                        if jc == 0:
                                            acc_eng.tensor_copy(eaccs[i], et)
                                        else:
                                            acc_eng.tensor_add(
                                                eaccs[i][:, ds(qoff, w)],
                                                eaccs[i][:, ds(qoff, w)],
                                                et[:, :w])
                                        if jc == 4 * i + 3:
                                            den_ps = pdd.tile([1, 512], F32,
                                                              name="dden")
                                            nc.tensor.matmul(den_ps, ones_kf,
                                                             eaccs[i],
                                                             start=True,
                                                             stop=True)
                                            lnt = pdt.tile([1, 512], F32,
                                                           name="dln")
                                            nc.scalar.activation(lnt, den_ps,
                                                                 AF.Ln)
                                            rk = pdt.tile([1, 512], F32R,
                                                          name="drk")
                                            nc.scalar.activation(rk, lnt,
                                                                 AF.Exp,
                                                                 scale=-1.0)
                                            bc = pdsc.tile([P, 512], F32,
                                                           name="psc")
                                            nc.tensor.matmul(bc, ones_b, rk,
                                                             start=True,
                                                             stop=True)
                                            bcs = pdt.tile([P, 512], F32,
                                                           name="dbcs")
                                            nc.vector.tensor_copy(bcs, bc)
                                            nc.vector.tensor_mul(
                                                o_sb[:, h, ts(i, 512)],
                                                ps_o[i], bcs)

                    if DEBUG:
                        nc.sync.dma_start(dbg_o, o_sb)
                    # ---------------- Phase F: o_proj partial -------------
                    with (
                        tc.tile_pool(name="pf", bufs=1) as pf,
                        tc.tile_pool(name="pf_r", bufs=2) as pfr,
                        tc.tile_pool(name="pf_ps", bufs=4, space="PSUM") as pfp,
                    ):
                        with nc.named_scope("phaseF"):
                            wo_sb = pf.tile([P, 4, HID], BF16)
                            nc.sync.dma_start(wo_sb, wo_in)
                            for tch in range(NTC):
                                orow = pfr.tile([P, HID], BF16, name="orow")
                                pss = [pfp.tile([P, 512], F32, name="fps")
                                       for _ in range(4)]
                                for hh in range(4):
                                    for ct in range(4):
                                        nc.tensor.matmul(
                                            pss[ct],
                                            o_sb[:, hh, ds(tch * P, P)],
                                            wo_sb[:, hh, ts(ct, 512)],
                                            start=(hh == 0), stop=(hh == 3),
                                        )
                                for ct in range(4):
                                    nc.vector.tensor_copy(
                                        orow[:, ts(ct, 512)], pss[ct])
                                nc.sync.dma_start(out_d[ds(tch * P, P), :],
                                                  orow)


_NC_CACHE = None


def _build_nc():
    global _NC_CACHE
    if _NC_CACHE is None:
        nc = bacc.Bacc("TRN2", target_bir_lowering=False, debug=False,
                       num_devices=8)
        with tile.TileContext(nc) as tc:
            _emit(tc)
        nc.compile()
        _NC_CACHE = nc
    return _NC_CACHE


def _shard_inputs(hidden_states, cos, sin, Wq_down, q_gamma, Wq_up,
                  Wkv_down, kv_gamma, Wkv_up, Wo):
    f32 = np.float32
    hid = np.asarray(hidden_states, dtype=f32)
    cos = np.asarray(cos, dtype=f32)
    sin = np.asarray(sin, dtype=f32)
    Wqd = np.asarray(Wq_down, dtype=f32)
    Wkd = np.asarray(Wkv_down, dtype=f32)
    qg = np.asarray(q_gamma, dtype=f32)
    kvg = np.asarray(kv_gamma, dtype=f32)
    Wqu = np.asarray(Wq_up, dtype=f32) * qg[None, :]
    Wku = np.asarray(Wkv_up, dtype=f32) * kvg[None, :]
    Wo = np.asarray(Wo, dtype=f32)

    # combined down-proj weight, kv-first: [ckv | kr | kr | rot | rot | q]
    WckvT = Wkd[:KVR].T                            # [HID, KVR]
    krope = Wkd[KVR:].T                            # [HID, 64]
    krot = np.concatenate([-krope[:, 32:], krope[:, :32]], 1)
    WqdT = Wqd.T                                   # [HID, QR]
    WdT = np.concatenate([WckvT, krope, krope, krot, krot, WqdT], 1)
    wd = np.ascontiguousarray(
        WdT.reshape(NHC, P, NFC, P).transpose(2, 1, 0, 3)).astype(NPBF)

    # rotate_half permutation for the q-rope head pairs
    prot = np.zeros((P, P), dtype=f32)
    for base in (0, 64):
        for t in range(32):
            prot[base + 32 + t, base + t] = -1.0
            prot[base + t, base + 32 + t] = 1.0
    prot = prot.astype(NPBF)

    per_batch = []
    for b in range(B):
        h_sw = np.ascontiguousarray(
            hid[b].T.reshape(NHC, P, S).transpose(1, 0, 2))  # [128, 16, S]
        cT = cos[b].T                               # [64, S]
        sT = sin[b].T
        cos2 = np.ascontiguousarray(np.concatenate([cT, cT], 0))
        sin2 = np.ascontiguousarray(np.concatenate([sT, sT], 0))
        per_batch.append((h_sw, cos2, sin2))

    per_group = []
    for g in range(4):
        bn, br = [], []
        for hl in range(4):
            h = 4 * g + hl
            blk = Wqu[h * QKD:(h + 1) * QKD]       # [192, QR]
            bn.append(blk[:NOPE])
            br.append(blk[NOPE:])
        cols = bn + [np.concatenate([br[0], br[1]], 0),
                     np.concatenate([br[2], br[3]], 0)]
        WquT = np.concatenate(cols, 0).T           # [QR, 768]
        wqup = np.ascontiguousarray(
            WquT.reshape(NQC, P, NDQ * P).transpose(1, 0, 2)).astype(NPBF)
        kb, vb = [], []
        for hl in range(4):
            h = 4 * g + hl
            blk = Wku[h * (NOPE + VD):(h + 1) * (NOPE + VD)]
            kb.append(blk[:NOPE])
            vb.append(blk[NOPE:])
        WkuT = np.concatenate(kb, 0).T             # [KVR, 512]
        WvuT = np.concatenate(vb, 0).T
        wkup = np.ascontiguousarray(
            WkuT.reshape(NKC, P, 512).transpose(1, 0, 2)).astype(NPBF)
        wvup = np.ascontiguousarray(
            WvuT.reshape(NKC, P, 512).transpose(1, 0, 2)).astype(NPBF)
        WoT = Wo[:, g * 512:(g + 1) * 512].T       # [512, HID]
        wo = np.ascontiguousarray(
            WoT.reshape(4, P, HID).transpose(1, 0, 2)).astype(NPBF)
        per_group.append((wqup, wkup, wvup, wo))

    in_maps = []
    for c in range(8):
        b, g = c // 4, c % 4
        h_sw, cos2, sin2 = per_batch[b]
        wqup, wkup, wvup, wo = per_group[g]
        in_maps.append({
            "hid": np.ascontiguousarray(
                h_sw[:, :, g * 512:(g + 1) * 512]).astype(NPBF),
            "cos2": cos2, "sin2": sin2, "wd": wd, "prot": prot,
            "wqup": wqup, "wkup": wkup, "wvup": wvup, "wo": wo,
        })
    return in_maps


def kernel(hidden_states, cos, sin, Wq_down, q_gamma, Wq_up,
           Wkv_down, kv_gamma, Wkv_up, Wo, _trace=False):
    nc = _build_nc()
    in_maps = _shard_inputs(hidden_states, cos, sin, Wq_down, q_gamma, Wq_up,
                            Wkv_down, kv_gamma, Wkv_up, Wo)
    res = run_bass_kernel_spmd(nc, in_maps, core_ids=list(range(8)),
                               trace=_trace)
    out = np.zeros((B, S, HID), dtype=np.float32)
    for c in range(8):
        out[c // 4] += np.asarray(res.results[c]["out"], dtype=np.float32)
    if _trace:
        kernel.last_results = res
    return out


# revision 18
# speedup vs baseline: 1.0348x; 1.0348x over previous
"""MLA (Multi-head Latent Attention) Bass/Tile kernel for 8 Trainium2 NeuronCores.

Problem: nn_MultiHeadLatentAttention_81707457839331
  B=2, S=2048, HID=2048, NH=16 heads, NOPE=128, ROPE=64, VD=128, QKD=192,
  KVR=512, QR=1536, fp32 in/out.

Sharding (single NEFF, SPMD on 8 cores):
  core c -> batch b = c//4, head group g = c%4 (4 heads each).
  The shared down-projection is SEQUENCE-PARALLEL inside each 4-core batch
  group: core g computes its 512-token slice of all 2304 latent features,
  then two AllGathers (kv chunks first, q-latent chunks second) replicate
  the full latents to the group. q_up/kv_up/attention/o_proj are
  head-sharded. Each core emits a partial o_proj output [S, HID]; the host
  sums the 4 partials per batch.

Perf notes vs the fp32r baseline (967us):
  - all matmul inputs are bf16 (same PE rate as f32r at moving dim 512, but
    1.0 cyc/row LDWEIGHTS instead of 1.5, half the DMA/SBUF footprint;
    numpy-sim puts the end-to-end rel err at ~5.5e-3 vs the 2e-2 gate),
  - loop orders reuse each stationary across all 4 token tiles, cutting
    LDWEIGHTS count ~3x (the baseline spent 484us in LDW),
  - RMSNorm partition reductions + softmax denominators run on the PE as
    ones-vector matmuls accumulated in PSUM (the baseline burned ~90us of
    GpSimd partition_all_reduce + 80us of DVE full-tile reciprocals),
  - per-token scales are broadcast to 128 partitions with a rank-1 PE
    matmul instead of gpsimd,
  - rotate_half for q-rope is a PE permutation matmul (host-built +-1
    matrix), dropping the 2 extra rot weight chunks from q_up,
  - causal mask is applied post-exp as a bf16 0/1 multiply on DVE,
  - kt/kre/kro/v/qT/attention-out all stay in SBUF between phases.
"""

import numpy as np
import ml_dtypes

import concourse.bass as bass
import concourse.bass_isa as bass_isa
import concourse.mybir as mybir
import concourse.tile as tile
from concourse import bacc
from concourse.bass import ds, ts
from concourse.bass_utils import run_bass_kernel_spmd

F32 = mybir.dt.float32
F32R = mybir.dt.float32r
BF16 = mybir.dt.bfloat16
AF = mybir.ActivationFunctionType
NPBF = ml_dtypes.bfloat16

B, S, HID, NH = 2, 2048, 2048, 16
NOPE, ROPE, VD = 128, 64, 128
QKD = NOPE + ROPE
KVR, QR = 512, 1536
EPS = 1e-6
SCALE = QKD ** (-0.5)
P = 128

NHC = HID // P            # 16 hidden chunks
NQC = QR // P             # 12 q-latent chunks
NKC = KVR // P            # 4 ckv chunks
NFC = 6 + NQC             # down-proj chunks: 4 ckv + rope-dup + rot-dup + 12 q
NDQ = 6                   # q_up output chunks: 4 nope + 2 rope-pairs
NTT = S // 512            # 4 token tiles of 512
NTC = S // P              # 16 token chunks of 128
RG = [[0, 1, 2, 3], [4, 5, 6, 7]]
DEBUG = False


def _emit(tc):
    nc = tc.nc
    hid_in = nc.dram_tensor("hid", [P, NHC, 512], BF16, kind="ExternalInput").ap()
    cos_in = nc.dram_tensor("cos2", [P, S], F32, kind="ExternalInput").ap()
    sin_in = nc.dram_tensor("sin2", [P, S], F32, kind="ExternalInput").ap()
    wd_in = nc.dram_tensor("wd", [NFC, P, NHC, P], BF16, kind="ExternalInput").ap()
    wqup_in = nc.dram_tensor("wqup", [P, NQC, NDQ * P], BF16, kind="ExternalInput").ap()
    wkup_in = nc.dram_tensor("wkup", [P, NKC, 512], BF16, kind="ExternalInput").ap()
    wvup_in = nc.dram_tensor("wvup", [P, NKC, 512], BF16, kind="ExternalInput").ap()
    wo_in = nc.dram_tensor("wo", [P, 4, HID], BF16, kind="ExternalInput").ap()
    prot_in = nc.dram_tensor("prot", [P, P], BF16, kind="ExternalInput").ap()
    out_d = nc.dram_tensor("out", [S, HID], BF16, kind="ExternalOutput").ap()
    if DEBUG:
        dbg_kv = nc.dram_tensor("dbg_kv", [P, 6, NTT, 512], BF16,
                                kind="ExternalOutput").ap()
        dbg_kt = nc.dram_tensor("dbg_kt", [P, 4, S], BF16,
                                kind="ExternalOutput").ap()
        dbg_kre = nc.dram_tensor("dbg_kre", [P, S], BF16,
                                 kind="ExternalOutput").ap()
        dbg_kro = nc.dram_tensor("dbg_kro", [P, S], BF16,
                                 kind="ExternalOutput").ap()
        dbg_v = nc.dram_tensor("dbg_v", [P, NTC, 512], BF16,
                               kind="ExternalOutput").ap()
        dbg_qt = nc.dram_tensor("dbg_qt", [P, 6, S], BF16,
                                kind="ExternalOutput").ap()
        dbg_o = nc.dram_tensor("dbg_o", [P, 4, S], BF16,
                               kind="ExternalOutput").ap()
        dbg_mask = nc.dram_tensor("dbg_mask", [P, 4, 512], BF16,
                                  kind="ExternalOutput").ap()

    with (
        tc.tile_pool(name="const", bufs=1) as constp,
        tc.tile_pool(name="dram", bufs=1, space="DRAM") as dramp,
    ):
        eps_kv = constp.tile([1, 1], F32)
        nc.vector.memset(eps_kv, EPS)
        eps_q = constp.tile([1, 1], F32)
        nc.vector.memset(eps_q, EPS / (SCALE * SCALE))
        ones_f32 = constp.tile([P, 1], F32)
        nc.vector.memset(ones_f32, 1.0)
        ones_kf = constp.tile([P, 1], F32R)       # colsum stationary
        nc.vector.tensor_copy(ones_kf, ones_f32)
        ones_b = constp.tile([1, P], F32R)        # rank-1 row broadcast
        ones_bf32 = constp.tile([1, P], F32)
        nc.vector.memset(ones_bf32, 1.0)
        nc.vector.tensor_copy(ones_b, ones_bf32)
        # causal 0/1 mask for the (narrowed) diagonal: mask[p,x] = x >= p
        mask0 = constp.tile([P, 512], BF16, name="mask0")
        nc.gpsimd.memset(mask0, 1.0)
        nc.gpsimd.affine_select(
            out=mask0, in_=mask0, pattern=[[1, 512]],
            compare_op=mybir.AluOpType.is_ge, fill=0.0,
            base=0, channel_multiplier=-1,
        )
        if DEBUG:
            for k in range(4):
                nc.sync.dma_start(dbg_mask[:, k, :], mask0)

        # AllGather bounce buffers (token-slice in, full-sequence out)
        ag_in = dramp.tile([NFC, P, 512], BF16)
        ag_kv = dramp.tile([NTT, 6, P, 512], BF16)
        ag_q = dramp.tile([NTT, NQC, P, 512], BF16)

        # ---------------- Phase A: seq-parallel down-projection ----------
        with (
            tc.tile_pool(name="pa", bufs=1) as pa,
            tc.tile_pool(name="pa_row", bufs=3) as par,
            tc.tile_pool(name="pa_ps", bufs=3, space="PSUM") as pap,
        ):
            with nc.named_scope("phaseA"):
                hid_sb = pa.tile([P, NHC, 512], BF16)
                nc.sync.dma_start(hid_sb, hid_in)
                wd_sb = pa.tile([P, NFC, NHC, P], BF16)
                for fc in range(NFC):
                    nc.sync.dma_start(wd_sb[:, fc], wd_in[fc])
                for fc in range(NFC):
                    ps = pap.tile([P, 512], F32, name="aps")
                    for hc in range(NHC):
                        nc.tensor.matmul(
                            ps, wd_sb[:, fc, hc, :], hid_sb[:, hc, :],
                            start=(hc == 0), stop=(hc == NHC - 1),
                        )
                    row = par.tile([P, 512], BF16, name="arow")
                    nc.vector.tensor_copy(row, ps)
                    nc.sync.dma_start(ag_in[fc], row)
                    if fc == 5:
                        nc.gpsimd.collective_compute(
                            "AllGather", mybir.AluOpType.bypass, RG,
                            ins=[ag_in[0:6].opt()], outs=[ag_kv[:].opt()],
                        )
                nc.gpsimd.collective_compute(
                    "AllGather", mybir.AluOpType.bypass, RG,
                    ins=[ag_in[6:NFC].opt()], outs=[ag_q[:].opt()],
                )

        # kt/kre/kro/v stay in SBUF from phase B through phase D
        with tc.tile_pool(name="pkv", bufs=1) as pkv:
            kt_sb = pkv.tile([P, 4, S], BF16)     # 4 heads k_nope.T
            kre_sb = pkv.tile([P, S], BF16)       # [k_roped; 0]
            kro_sb = pkv.tile([P, S], BF16)       # [0; k_roped]
            v_sb = pkv.tile([P, NTC, 512], BF16)  # V in [token, 4*VD]
            cos_sb = pkv.tile([P, S], F32)
            sin_sb = pkv.tile([P, S], F32)
            nc.sync.dma_start(cos_sb, cos_in)
            nc.sync.dma_start(sin_sb, sin_in)
            nc.vector.memset(kre_sb[64:128, :], 0.0)
            nc.vector.memset(kro_sb[0:64, :], 0.0)

            # ---------------- Phase B: kv norm + rope-k + kv_up ----------
            with (
                tc.tile_pool(name="pb", bufs=1) as pb,
                tc.tile_pool(name="pb_t", bufs=3) as pbt,
                tc.tile_pool(name="pb_st", bufs=2, space="PSUM") as pbst,
                tc.tile_pool(name="pb_bc", bufs=1, space="PSUM") as pbbc,
                tc.tile_pool(name="pb_ps", bufs=4, space="PSUM") as pbps,
            ):
                with nc.named_scope("phaseB"):
                    kv_sb = pb.tile([P, 6, NTT, 512], BF16)
                    for tt in range(NTT):
                        for fc in range(6):
                            nc.sync.dma_start(kv_sb[:, fc, tt, :],
                                              ag_kv[tt, fc])
                    if DEBUG:
                        nc.sync.dma_start(dbg_kv, kv_sb)
                    kvn = pb.tile([P, NKC, NTT, 512], BF16)
                    for tt in range(NTT):
                        eng = nc.vector if tt % 2 == 0 else nc.gpsimd
                        acc = pbt.tile([P, 512], F32R, name="bacc")
                        nc.scalar.square(acc, kv_sb[:, 0, tt, :])
                        for fc in range(1, NKC):
                            sq = pbt.tile([P, 512], F32, name="bsq")
                            nc.scalar.square(sq, kv_sb[:, fc, tt, :])
                            eng.tensor_add(acc, acc, sq)
                        st = pbst.tile([1, 512], F32, name="bst")
                        nc.tensor.matmul(st, ones_kf, acc, start=True,
                                         stop=True)
                        srt = pbt.tile([1, 512], F32, name="bsrt")
                        nc.scalar.activation(srt, st, AF.Ln, bias=eps_kv,
                                             scale=1.0 / KVR)
                        rk = pbt.tile([1, 512], F32R, name="brk")
                        nc.scalar.activation(rk, srt, AF.Exp, scale=-0.5)
                        bc = pbbc.tile([P, 512], F32, name="bbc")
                        nc.tensor.matmul(bc, ones_b, rk, start=True, stop=True)
                        bcs = pbt.tile([P, 512], F32, name="bbcs")
                        nc.vector.tensor_copy(bcs, bc)
                        for fc in range(NKC):
                            eng.tensor_mul(kvn[:, fc, tt, :],
                                           kv_sb[:, fc, tt, :], bcs)
                        # shared rope key (chunk 4 = [kr|kr], 5 = [rot|rot])
                        t1 = pbt.tile([P, 512], F32, name="bt1")
                        nc.vector.tensor_mul(t1, kv_sb[:, 4, tt, :],
                                             cos_sb[:, ts(tt, 512)])
                        t2 = pbt.tile([P, 512], F32, name="bt2")
                        nc.vector.tensor_mul(t2, kv_sb[:, 5, tt, :],
                                             sin_sb[:, ts(tt, 512)])
                        nc.vector.tensor_add(t1, t1, t2)
                        nc.vector.tensor_copy(kre_sb[0:64, ts(tt, 512)],
                                              t1[0:64, :])
                        nc.vector.tensor_copy(kro_sb[64:128, ts(tt, 512)],
                                              t1[64:128, :])
                    # kv_up: k-heads (stationary reused over token tiles)
                    wk_sb = pb.tile([P, NKC, 512], BF16)
                    nc.sync.dma_start(wk_sb, wkup_in)
                    wv_sb = pb.tile([P, NKC, 512], BF16)
                    nc.sync.dma_start(wv_sb, wvup_in)
                    for d in range(4):
                        pss = [pbps.tile([P, 512], F32, name="bkp")
                               for _ in range(NTT)]
                        for fc in range(NKC):
                            for tt in range(NTT):
                                nc.tensor.matmul(
                                    pss[tt], wk_sb[:, fc, ds(d * P, P)],
                                    kvn[:, fc, tt, :],
                                    start=(fc == 0), stop=(fc == NKC - 1),
                                )
                        for tt in range(NTT):
                            nc.vector.tensor_copy(kt_sb[:, d, ts(tt, 512)],
                                                  pss[tt])
                    # V in [token, head*vd]
                    for tch in range(NTC):
                        ps = pbps.tile([P, 512], F32, name="bkp")
                        for fc in range(NKC):
                            nc.tensor.matmul(
                                ps, kvn[:, fc, tch // 4, ds((tch % 4) * P, P)],
                                wv_sb[:, fc, :],
                                start=(fc == 0), stop=(fc == NKC - 1),
                            )
                        nc.vector.tensor_copy(v_sb[:, tch, :], ps)

            if DEBUG:
                nc.sync.dma_start(dbg_kt, kt_sb)
                nc.sync.dma_start(dbg_kre, kre_sb)
                nc.sync.dma_start(dbg_kro, kro_sb)
                nc.sync.dma_start(dbg_v, v_sb)

            # qT lives in SBUF from phase C through phase D
            with tc.tile_pool(name="pq", bufs=1) as pq:
                qT = pq.tile([P, 6, S], BF16)  # 4 nope + 2 roped pairs

                # ------------- Phase C: q_up + rope-q + q-norm -----------
                with (
                    tc.tile_pool(name="pc", bufs=1) as pc,
                    tc.tile_pool(name="pc_t", bufs=2) as pct,
                    tc.tile_pool(name="pc_st", bufs=2, space="PSUM") as pcst,
                    tc.tile_pool(name="pc_bc", bufs=2, space="PSUM") as pcbc,
                    tc.tile_pool(name="pc_ps", bufs=4, space="PSUM") as pcps,
                ):
                    with nc.named_scope("phaseC"):
                        wq_sb = pc.tile([P, NQC, NDQ * P], BF16)
                        nc.sync.dma_start(wq_sb, wqup_in)
                        prot_sb = pc.tile([P, P], BF16)
                        nc.sync.dma_start(prot_sb, prot_in)
                        latq = pc.tile([P, NQC, NTT, 512], BF16)
                        for tt in range(NTT):
                            for fc in range(NQC):
                                nc.sync.dma_start(latq[:, fc, tt, :],
                                                  ag_q[tt, fc])
                        rqs = []
                        for tt in range(NTT):
                            eng = nc.vector if tt % 2 == 0 else nc.gpsimd
                            acc = pct.tile([P, 512], F32R, name="cacc")
                            nc.scalar.square(acc, latq[:, 0, tt, :])
                            for fc in range(1, NQC):
                                sq = pct.tile([P, 512], F32, name="csq")
                                nc.scalar.square(sq, latq[:, fc, tt, :])
                                eng.tensor_add(acc, acc, sq)
                            st = pcst.tile([1, 512], F32, name="cst")
                            nc.tensor.matmul(st, ones_kf, acc,
                                             start=True, stop=True)
                            srt = pct.tile([1, 512], F32, name="csrt")
                            nc.scalar.activation(srt, st, AF.Ln, bias=eps_q,
                                                 scale=1.0 / (QR * SCALE * SCALE))
                            rk = pct.tile([1, 512], F32R, name="crk")
                            nc.scalar.activation(rk, srt, AF.Exp, scale=-0.5)
                            bc = pcbc.tile([P, 512], F32, name="cbc")
                            nc.tensor.matmul(bc, ones_b, rk,
                                             start=True, stop=True)
                            rq = pc.tile([P, 512], F32, name=f"crq{tt}")
                            nc.vector.tensor_copy(rq, bc)
                            rqs.append(rq)
                        for d in range(NDQ):
                            pss = [pcps.tile([P, 512], F32, name="cqp")
                                   for _ in range(NTT)]
                            for fc in range(NQC):
                                for tt in range(NTT):
                                    nc.tensor.matmul(
                                        pss[tt], wq_sb[:, fc, ds(d * P, P)],
                                        latq[:, fc, tt, :],
                                        start=(fc == 0), stop=(fc == NQC - 1),
                                    )
                            if d < 4:
                                for tt in range(NTT):
                                    nc.vector.tensor_mul(
                                        qT[:, d, ts(tt, 512)], pss[tt],
                                        rqs[tt])
                            else:
                                # roped pair: rotate_half via PE permutation
                                for tt in range(NTT):
                                    qr = pct.tile([P, 512], BF16, name="cqr")
                                    nc.vector.tensor_copy(qr, pss[tt])
                                    rt = pcbc.tile([P, 512], F32, name="cbc")
                                    nc.tensor.matmul(rt, prot_sb, qr,
                                                     start=True, stop=True)
                                    t1 = pct.tile([P, 512], F32, name="ct1")
                                    nc.vector.tensor_mul(
                                        t1, qr, cos_sb[:, ts(tt, 512)])
                                    t2 = pct.tile([P, 512], F32, name="ct2")
                                    nc.vector.tensor_mul(
                                        t2, rt, sin_sb[:, ts(tt, 512)])
                                    nc.vector.tensor_add(t1, t1, t2)
                                    nc.vector.tensor_mul(
                                        qT[:, d, ts(tt, 512)], t1, rqs[tt])

                if DEBUG:
                    nc.sync.dma_start(dbg_qt, qT)
                # ---------------- Phase D: attention ---------------------
                with tc.tile_pool(name="po", bufs=1) as po:
                    o_sb = po.tile([P, 4, S], BF16)
                    with (
                        tc.tile_pool(name="pd_e", bufs=6) as pde,
                        tc.tile_pool(name="pd_a", bufs=8) as pda,
                        tc.tile_pool(name="pd_t", bufs=4) as pdt,
                        tc.tile_pool(name="pd_sc", bufs=3, space="PSUM") as pdsc,
                        tc.tile_pool(name="pd_o", bufs=4, space="PSUM") as pdo,
                        tc.tile_pool(name="pd_den", bufs=1, space="PSUM") as pdd,
                    ):
                        with nc.named_scope("phaseD"):
                            for h in range(4):
                                krop = kre_sb if h % 2 == 0 else kro_sb
                                acc_eng = nc.vector if h % 2 == 0 else nc.gpsimd
                                qp = qT[:, 4 + h // 2, :]
                                ps_o = [pdo.tile([P, 512], F32, name="pso")
                                        for i in range(4)]
                                eaccs = [pda.tile([P, 512], F32R, name="eacc")
                                         for i in range(4)]
                                for jc in range(NTC):
                                    imin = jc // 4
                                    ets = {}
                                    for i in range(imin, 4):
                                        qoff = (jc % 4) * P if i == imin else 0
                                        w = 512 - qoff
                                        ps_sc = pdsc.tile([P, 512], F32,
                                                          name="psc")
                                        nc.tensor.matmul(
                                            ps_sc[:, :w],
                                            kt_sb[:, h, ds(jc * P, P)],
                                            qT[:, h, ds(i * 512 + qoff, w)],
                                            start=True, stop=False)
                                        nc.tensor.matmul(
                                            ps_sc[:, :w],
                                            krop[:, ds(jc * P, P)],
                                            qp[:, ds(i * 512 + qoff, w)],
                                            start=False, stop=True)
                                        et = pde.tile([P, 512], BF16,
                                                      name="et")
                                        nc.scalar.activation(et[:, :w],
                                                             ps_sc[:, :w],
                                                             AF.Exp)
                                        if i == imin:
                                            # beyond col 128 the narrowed
                                            # diag block is fully valid
                                            nc.vector.tensor_mul(
                                                et[:, :P], et[:, :P],
                                                mask0[:, :P])
                                        ets[i] = (et, qoff, w)
                                    for i in range(imin, 4):
                                        et, qoff, w = ets[i]
                                        nc.tensor.matmul(
                                            ps_o[i][:, ds(qoff, w)],
                                            v_sb[:, jc, ds(h * P, P)],
                                            et[:, :w],
                                            start=(jc == 0),
                                            stop=(jc == 4 * i + 3))
                                        if jc == 0:
                                            acc_eng.tensor_copy(eaccs[i], et)
                                        else:
                                            acc_eng.tensor_add(
                                                eaccs[i][:, ds(qoff, w)],
                                                eaccs[i][:, ds(qoff, w)],
                                                et[:, :w])
                                        if jc == 4 * i + 3:
                                            den_ps = pdd.tile([1, 512], F32,
                                                              name="dden")
                                            nc.tensor.matmul(den_ps, ones_kf,
                                                             eaccs[i],
                                                             start=True,
                                                             stop=True)
                                            lnt = pdt.tile([1, 512], F32,
                                                           name="dln")
                                            nc.scalar.activation(lnt, den_ps,
                                                                 AF.Ln)
                                            rk = pdt.tile([1, 512], F32R,
                                                          name="drk")
                                            nc.scalar.activation(rk, lnt,
                                                                 AF.Exp,
                                                                 scale=-1.0)
                                            bc = pdsc.tile([P, 512], F32,
                                                           name="psc")
                                            nc.tensor.matmul(bc, ones_b, rk,
                                                             start=True,
                                                             stop=True)
                                            bcs = pdt.tile([P, 512], F32,
                                                           name="dbcs")
                                            nc.vector.tensor_copy(bcs, bc)
                                            nc.vector.tensor_mul(
                                                o_sb[:, h, ts(i, 512)],
                                                ps_o[i], bcs)

                    if DEBUG:
                        nc.sync.dma_start(dbg_o, o_sb)
                    # ---------------- Phase F: o_proj partial -------------
                    with (
                        tc.tile_pool(name="pf", bufs=1) as pf,
                        tc.tile_pool(name="pf_r", bufs=2) as pfr,
                        tc.tile_pool(name="pf_ps", bufs=4, space="PSUM") as pfp,
                    ):
                        with nc.named_scope("phaseF"):
                            wo_sb = pf.tile([P, 4, HID], BF16)
                            nc.sync.dma_start(wo_sb, wo_in)
                            for tch in range(NTC):
                                orow = pfr.tile([P, HID], BF16, name="orow")
                                pss = [pfp.tile([P, 512], F32, name="fps")
                                       for _ in range(4)]
                                for hh in range(4):
                                    for ct in range(4):
                                        nc.tensor.matmul(
                                            pss[ct],
                                            o_sb[:, hh, ds(tch * P, P)],
                                            wo_sb[:, hh, ts(ct, 512)],
                                            start=(hh == 0), stop=(hh == 3),
                                        )
                                for ct in range(4):
                                    nc.vector.tensor_copy(
                                        orow[:, ts(ct, 512)], pss[ct])
                                nc.sync.dma_start(out_d[ds(tch * P, P), :],
                                                  orow)


_NC_CACHE = None


def _build_nc():
    global _NC_CACHE
    if _NC_CACHE is None:
        nc = bacc.Bacc("TRN2", target_bir_lowering=False, debug=False,
                       num_devices=8)
        with tile.TileContext(nc) as tc:
            _emit(tc)
        nc.compile()
        _NC_CACHE = nc
    return _NC_CACHE


def _shard_inputs(hidden_states, cos, sin, Wq_down, q_gamma, Wq_up,
                  Wkv_down, kv_gamma, Wkv_up, Wo):
    f32 = np.float32
    hid = np.asarray(hidden_states, dtype=f32)
    cos = np.asarray(cos, dtype=f32)
    sin = np.asarray(sin, dtype=f32)
    Wqd = np.asarray(Wq_down, dtype=f32)
    Wkd = np.asarray(Wkv_down, dtype=f32)
    qg = np.asarray(q_gamma, dtype=f32)
    kvg = np.asarray(kv_gamma, dtype=f32)
    Wqu = np.asarray(Wq_up, dtype=f32) * qg[None, :]
    Wku = np.asarray(Wkv_up, dtype=f32) * kvg[None, :]
    Wo = np.asarray(Wo, dtype=f32)

    # combined down-proj weight, kv-first: [ckv | kr | kr | rot | rot | q]
    WckvT = Wkd[:KVR].T                            # [HID, KVR]
    krope = Wkd[KVR:].T                            # [HID, 64]
    krot = np.concatenate([-krope[:, 32:], krope[:, :32]], 1)
    WqdT = Wqd.T                                   # [HID, QR]
    WdT = np.concatenate([WckvT, krope, krope, krot, krot, WqdT], 1)
    wd = np.ascontiguousarray(
        WdT.reshape(NHC, P, NFC, P).transpose(2, 1, 0, 3)).astype(NPBF)

    # rotate_half permutation for the q-rope head pairs
    prot = np.zeros((P, P), dtype=f32)
    for base in (0, 64):
        for t in range(32):
            prot[base + 32 + t, base + t] = -1.0
            prot[base + t, base + 32 + t] = 1.0
    prot = prot.astype(NPBF)

    per_batch = []
    for b in range(B):
        h_sw = np.ascontiguousarray(
            hid[b].T.reshape(NHC, P, S).transpose(1, 0, 2))  # [128, 16, S]
        cT = cos[b].T                               # [64, S]
        sT = sin[b].T
        cos2 = np.ascontiguousarray(np.concatenate([cT, cT], 0))
        sin2 = np.ascontiguousarray(np.concatenate([sT, sT], 0))
        per_batch.append((h_sw, cos2, sin2))

    per_group = []
    for g in range(4):
        bn, br = [], []
        for hl in range(4):
            h = 4 * g + hl
            blk = Wqu[h * QKD:(h + 1) * QKD]       # [192, QR]
            bn.append(blk[:NOPE])
            br.append(blk[NOPE:])
        cols = bn + [np.concatenate([br[0], br[1]], 0),
                     np.concatenate([br[2], br[3]], 0)]
        WquT = np.concatenate(cols, 0).T           # [QR, 768]
        wqup = np.ascontiguousarray(
            WquT.reshape(NQC, P, NDQ * P).transpose(1, 0, 2)).astype(NPBF)
        kb, vb = [], []
        for hl in range(4):
            h = 4 * g + hl
            blk = Wku[h * (NOPE + VD):(h + 1) * (NOPE + VD)]
            kb.append(blk[:NOPE])
            vb.append(blk[NOPE:])
        WkuT = np.concatenate(kb, 0).T             # [KVR, 512]
        WvuT = np.concatenate(vb, 0).T
        wkup = np.ascontiguousarray(
            WkuT.reshape(NKC, P, 512).transpose(1, 0, 2)).astype(NPBF)
        wvup = np.ascontiguousarray(
            WvuT.reshape(NKC, P, 512).transpose(1, 0, 2)).astype(NPBF)
        WoT = Wo[:, g * 512:(g + 1) * 512].T       # [512, HID]
        wo = np.ascontiguousarray(
            WoT.reshape(4, P, HID).transpose(1, 0, 2)).astype(NPBF)
        per_group.append((wqup, wkup, wvup, wo))

    in_maps = []
    for c in range(8):
        b, g = c // 4, c % 4
        h_sw, cos2, sin2 = per_batch[b]
        wqup, wkup, wvup, wo = per_group[g]
        in_maps.append({
            "hid": np.ascontiguousarray(
                h_sw[:, :, g * 512:(g + 1) * 512]).astype(NPBF),
            "cos2": cos2, "sin2": sin2, "wd": wd, "prot": prot,
            "wqup": wqup, "wkup": wkup, "wvup": wvup, "wo": wo,
        })
    return in_maps


def kernel(hidden_states, cos, sin, Wq_down, q_gamma, Wq_up,
           Wkv_down, kv_gamma, Wkv_up, Wo, _trace=False):
    nc = _build_nc()
    in_maps = _shard_inputs(hidden_states, cos, sin, Wq_down, q_gamma, Wq_up,
                            Wkv_down, kv_gamma, Wkv_up, Wo)
    res = run_bass_kernel_spmd(nc, in_maps, core_ids=list(range(8)),
                               trace=_trace)
    out = np.zeros((B, S, HID), dtype=np.float32)
    for c in range(8):
        out[c // 4] += np.asarray(res.results[c]["out"], dtype=np.float32)
    if _trace:
        kernel.last_results = res
    return out


# revision 19
# speedup vs baseline: 1.0499x; 1.0146x over previous
"""MLA (Multi-head Latent Attention) Bass/Tile kernel for 8 Trainium2 NeuronCores.

Problem: nn_MultiHeadLatentAttention_81707457839331
  B=2, S=2048, HID=2048, NH=16 heads, NOPE=128, ROPE=64, VD=128, QKD=192,
  KVR=512, QR=1536, fp32 in/out.

Sharding (single NEFF, SPMD on 8 cores):
  core c -> batch b = c//4, head group g = c%4 (4 heads each).
  The shared down-projection is SEQUENCE-PARALLEL inside each 4-core batch
  group: core g computes its 512-token slice of all 2304 latent features,
  then two AllGathers (kv chunks first, q-latent chunks second) replicate
  the full latents to the group. q_up/kv_up/attention/o_proj are
  head-sharded. Each core emits a partial o_proj output [S, HID]; the host
  sums the 4 partials per batch.

Perf notes vs the fp32r baseline (967us):
  - all matmul inputs are bf16 (same PE rate as f32r at moving dim 512, but
    1.0 cyc/row LDWEIGHTS instead of 1.5, half the DMA/SBUF footprint;
    numpy-sim puts the end-to-end rel err at ~5.5e-3 vs the 2e-2 gate),
  - loop orders reuse each stationary across all 4 token tiles, cutting
    LDWEIGHTS count ~3x (the baseline spent 484us in LDW),
  - RMSNorm partition reductions + softmax denominators run on the PE as
    ones-vector matmuls accumulated in PSUM (the baseline burned ~90us of
    GpSimd partition_all_reduce + 80us of DVE full-tile reciprocals),
  - per-token scales are broadcast to 128 partitions with a rank-1 PE
    matmul instead of gpsimd,
  - rotate_half for q-rope is a PE permutation matmul (host-built +-1
    matrix), dropping the 2 extra rot weight chunks from q_up,
  - causal mask is applied post-exp as a bf16 0/1 multiply on DVE,
  - kt/kre/kro/v/qT/attention-out all stay in SBUF between phases.
"""

import numpy as np
import ml_dtypes

import concourse.bass as bass
import concourse.bass_isa as bass_isa
import concourse.mybir as mybir
import concourse.tile as tile
from concourse import bacc
from concourse.bass import ds, ts
from concourse.bass_utils import run_bass_kernel_spmd

F32 = mybir.dt.float32
F32R = mybir.dt.float32r
BF16 = mybir.dt.bfloat16
AF = mybir.ActivationFunctionType
NPBF = ml_dtypes.bfloat16

B, S, HID, NH = 2, 2048, 2048, 16
NOPE, ROPE, VD = 128, 64, 128
QKD = NOPE + ROPE
KVR, QR = 512, 1536
EPS = 1e-6
SCALE = QKD ** (-0.5)
P = 128

NHC = HID // P            # 16 hidden chunks
NQC = QR // P             # 12 q-latent chunks
NKC = KVR // P            # 4 ckv chunks
NFC = 6 + NQC             # down-proj chunks: 4 ckv + rope-dup + rot-dup + 12 q
NDQ = 6                   # q_up output chunks: 4 nope + 2 rope-pairs
NTT = S // 512            # 4 token tiles of 512
NTC = S // P              # 16 token chunks of 128
RG = [[0, 1, 2, 3], [4, 5, 6, 7]]
DEBUG = False


def _emit(tc):
    nc = tc.nc
    hid_in = nc.dram_tensor("hid", [P, NHC, 512], BF16, kind="ExternalInput").ap()
    cos_in = nc.dram_tensor("cos2", [P, S], F32, kind="ExternalInput").ap()
    sin_in = nc.dram_tensor("sin2", [P, S], F32, kind="ExternalInput").ap()
    wd_in = nc.dram_tensor("wd", [NFC, P, NHC, P], BF16, kind="ExternalInput").ap()
    wqup_in = nc.dram_tensor("wqup", [P, NQC, NDQ * P], BF16, kind="ExternalInput").ap()
    wkup_in = nc.dram_tensor("wkup", [P, NKC, 512], BF16, kind="ExternalInput").ap()
    wvup_in = nc.dram_tensor("wvup", [P, NKC, 512], BF16, kind="ExternalInput").ap()
    wo_in = nc.dram_tensor("wo", [P, 4, HID], BF16, kind="ExternalInput").ap()
    prot_in = nc.dram_tensor("prot", [P, P], BF16, kind="ExternalInput").ap()
    out_d = nc.dram_tensor("out", [S, HID], BF16, kind="ExternalOutput").ap()
    if DEBUG:
        dbg_kv = nc.dram_tensor("dbg_kv", [P, 6, NTT, 512], BF16,
                                kind="ExternalOutput").ap()
        dbg_kt = nc.dram_tensor("dbg_kt", [P, 4, S], BF16,
                                kind="ExternalOutput").ap()
        dbg_kre = nc.dram_tensor("dbg_kre", [P, S], BF16,
                                 kind="ExternalOutput").ap()
        dbg_kro = nc.dram_tensor("dbg_kro", [P, S], BF16,
                                 kind="ExternalOutput").ap()
        dbg_v = nc.dram_tensor("dbg_v", [P, NTC, 512], BF16,
                               kind="ExternalOutput").ap()
        dbg_qt = nc.dram_tensor("dbg_qt", [P, 6, S], BF16,
                                kind="ExternalOutput").ap()
        dbg_o = nc.dram_tensor("dbg_o", [P, 4, S], BF16,
                               kind="ExternalOutput").ap()
        dbg_mask = nc.dram_tensor("dbg_mask", [P, 4, 512], BF16,
                                  kind="ExternalOutput").ap()

    with (
        tc.tile_pool(name="const", bufs=1) as constp,
        tc.tile_pool(name="dram", bufs=1, space="DRAM") as dramp,
    ):
        eps_kv = constp.tile([1, 1], F32)
        nc.vector.memset(eps_kv, EPS)
        eps_q = constp.tile([1, 1], F32)
        nc.vector.memset(eps_q, EPS / (SCALE * SCALE))
        ones_f32 = constp.tile([P, 1], F32)
        nc.vector.memset(ones_f32, 1.0)
        ones_kf = constp.tile([P, 1], F32R)       # colsum stationary
        nc.vector.tensor_copy(ones_kf, ones_f32)
        ones_b = constp.tile([1, P], F32R)        # rank-1 row broadcast
        ones_bf32 = constp.tile([1, P], F32)
        nc.vector.memset(ones_bf32, 1.0)
        nc.vector.tensor_copy(ones_b, ones_bf32)
        # causal 0/1 mask for the (narrowed) diagonal: mask[p,x] = x >= p
        mask0 = constp.tile([P, 512], BF16, name="mask0")
        nc.gpsimd.memset(mask0, 1.0)
        nc.gpsimd.affine_select(
            out=mask0, in_=mask0, pattern=[[1, 512]],
            compare_op=mybir.AluOpType.is_ge, fill=0.0,
            base=0, channel_multiplier=-1,
        )
        if DEBUG:
            for k in range(4):
                nc.sync.dma_start(dbg_mask[:, k, :], mask0)

        # AllGather bounce buffers (token-slice in, full-sequence out)
        ag_in = dramp.tile([NFC, P, 512], BF16)
        ag_kv = dramp.tile([NTT, 6, P, 512], BF16)
        ag_q = dramp.tile([NTT, NQC, P, 512], BF16)

        # ---------------- Phase A: seq-parallel down-projection ----------
        with (
            tc.tile_pool(name="pa", bufs=1) as pa,
            tc.tile_pool(name="pa_row", bufs=8) as par,
            tc.tile_pool(name="pa_ps", bufs=4, space="PSUM") as pap,
        ):
            with nc.named_scope("phaseA"):
                hid_sb = pa.tile([P, NHC, 512], BF16)
                wd_sb = pa.tile([P, NFC, NHC, P], BF16)
                nc.sync.dma_start(hid_sb[:, 0:4, :], hid_in[:, 0:4, :])
                nc.sync.dma_start(wd_sb[:, 0], wd_in[0])
                nc.sync.dma_start(hid_sb[:, 4:, :], hid_in[:, 4:, :])
                for fc in range(1, NFC):
                    nc.sync.dma_start(wd_sb[:, fc], wd_in[fc])
                for fc in range(NFC):
                    ps = pap.tile([P, 512], F32, name="aps")
                    for hc in range(NHC):
                        nc.tensor.matmul(
                            ps, wd_sb[:, fc, hc, :], hid_sb[:, hc, :],
                            start=(hc == 0), stop=(hc == NHC - 1),
                        )
                    row = par.tile([P, 512], BF16, name="arow")
                    nc.vector.tensor_copy(row, ps)
                    nc.sync.dma_start(ag_in[fc], row)
                    if fc == 5:
                        nc.gpsimd.collective_compute(
                            "AllGather", mybir.AluOpType.bypass, RG,
                            ins=[ag_in[0:6].opt()], outs=[ag_kv[:].opt()],
                        )

        # kt/kre/kro/v stay in SBUF from phase B through phase D
        with tc.tile_pool(name="pkv", bufs=1) as pkv:
            kt_sb = pkv.tile([P, 4, S], BF16)     # 4 heads k_nope.T
            kre_sb = pkv.tile([P, S], BF16)       # [k_roped; 0]
            kro_sb = pkv.tile([P, S], BF16)       # [0; k_roped]
            v_sb = pkv.tile([P, NTC, 512], BF16)  # V in [token, 4*VD]
            cos_sb = pkv.tile([P, S], F32)
            sin_sb = pkv.tile([P, S], F32)
            nc.sync.dma_start(cos_sb, cos_in)
            nc.sync.dma_start(sin_sb, sin_in)
            nc.vector.memset(kre_sb[64:128, :], 0.0)
            nc.vector.memset(kro_sb[0:64, :], 0.0)

            # ---------------- Phase B: kv norm + rope-k + kv_up ----------
            with (
                tc.tile_pool(name="pb", bufs=1) as pb,
                tc.tile_pool(name="pb_t", bufs=3) as pbt,
                tc.tile_pool(name="pb_st", bufs=2, space="PSUM") as pbst,
                tc.tile_pool(name="pb_bc", bufs=1, space="PSUM") as pbbc,
                tc.tile_pool(name="pb_ps", bufs=4, space="PSUM") as pbps,
            ):
                with nc.named_scope("phaseB"):
                    kv_sb = pb.tile([P, 6, NTT, 512], BF16)
                    for tt in range(NTT):
                        for fc in range(6):
                            nc.sync.dma_start(kv_sb[:, fc, tt, :],
                                              ag_kv[tt, fc])
                    if DEBUG:
                        nc.sync.dma_start(dbg_kv, kv_sb)
                    kvn = pb.tile([P, NKC, NTT, 512], BF16)
                    for tt in range(NTT):
                        eng = nc.vector if tt % 2 == 0 else nc.gpsimd
                        acc = pbt.tile([P, 512], F32R, name="bacc")
                        nc.scalar.square(acc, kv_sb[:, 0, tt, :])
                        for fc in range(1, NKC):
                            sq = pbt.tile([P, 512], F32, name="bsq")
                            nc.scalar.square(sq, kv_sb[:, fc, tt, :])
                            eng.tensor_add(acc, acc, sq)
                        st = pbst.tile([1, 512], F32, name="bst")
                        nc.tensor.matmul(st, ones_kf, acc, start=True,
                                         stop=True)
                        srt = pbt.tile([1, 512], F32, name="bsrt")
                        nc.scalar.activation(srt, st, AF.Ln, bias=eps_kv,
                                             scale=1.0 / KVR)
                        rk = pbt.tile([1, 512], F32R, name="brk")
                        nc.scalar.activation(rk, srt, AF.Exp, scale=-0.5)
                        bc = pbbc.tile([P, 512], F32, name="bbc")
                        nc.tensor.matmul(bc, ones_b, rk, start=True, stop=True)
                        bcs = pbt.tile([P, 512], F32, name="bbcs")
                        nc.vector.tensor_copy(bcs, bc)
                        for fc in range(NKC):
                            eng.tensor_mul(kvn[:, fc, tt, :],
                                           kv_sb[:, fc, tt, :], bcs)
                        # shared rope key (chunk 4 = [kr|kr], 5 = [rot|rot])
                        t1 = pbt.tile([P, 512], F32, name="bt1")
                        nc.vector.tensor_mul(t1, kv_sb[:, 4, tt, :],
                                             cos_sb[:, ts(tt, 512)])
                        t2 = pbt.tile([P, 512], F32, name="bt2")
                        nc.vector.tensor_mul(t2, kv_sb[:, 5, tt, :],
                                             sin_sb[:, ts(tt, 512)])
                        nc.vector.tensor_add(t1, t1, t2)
                        nc.vector.tensor_copy(kre_sb[0:64, ts(tt, 512)],
                                              t1[0:64, :])
                        nc.vector.tensor_copy(kro_sb[64:128, ts(tt, 512)],
                                              t1[64:128, :])
                    nc.gpsimd.collective_compute(
                        "AllGather", mybir.AluOpType.bypass, RG,
                        ins=[ag_in[6:NFC].opt()], outs=[ag_q[:].opt()],
                    )
                    # kv_up: k-heads (stationary reused over token tiles)
                    wk_sb = pb.tile([P, NKC, 512], BF16)
                    nc.sync.dma_start(wk_sb, wkup_in)
                    wv_sb = pb.tile([P, NKC, 512], BF16)
                    nc.sync.dma_start(wv_sb, wvup_in)
                    for d in range(4):
                        pss = [pbps.tile([P, 512], F32, name="bkp")
                               for _ in range(NTT)]
                        for fc in range(NKC):
                            for tt in range(NTT):
                                nc.tensor.matmul(
                                    pss[tt], wk_sb[:, fc, ds(d * P, P)],
                                    kvn[:, fc, tt, :],
                                    start=(fc == 0), stop=(fc == NKC - 1),
                                )
                        for tt in range(NTT):
                            nc.vector.tensor_copy(kt_sb[:, d, ts(tt, 512)],
                                                  pss[tt])
                    # V in [token, head*vd]
                    for tch in range(NTC):
                        ps = pbps.tile([P, 512], F32, name="bkp")
                        for fc in range(NKC):
                            nc.tensor.matmul(
                                ps, kvn[:, fc, tch // 4, ds((tch % 4) * P, P)],
                                wv_sb[:, fc, :],
                                start=(fc == 0), stop=(fc == NKC - 1),
                            )
                        nc.vector.tensor_copy(v_sb[:, tch, :], ps)

            if DEBUG:
                nc.sync.dma_start(dbg_kt, kt_sb)
                nc.sync.dma_start(dbg_kre, kre_sb)
                nc.sync.dma_start(dbg_kro, kro_sb)
                nc.sync.dma_start(dbg_v, v_sb)

            # qT lives in SBUF from phase C through phase D
            with tc.tile_pool(name="pq", bufs=1) as pq:
                qT = pq.tile([P, 6, S], BF16)  # 4 nope + 2 roped pairs

                # ------------- Phase C: q_up + rope-q + q-norm -----------
                with (
                    tc.tile_pool(name="pc", bufs=1) as pc,
                    tc.tile_pool(name="pc_t", bufs=2) as pct,
                    tc.tile_pool(name="pc_st", bufs=2, space="PSUM") as pcst,
                    tc.tile_pool(name="pc_bc", bufs=2, space="PSUM") as pcbc,
                    tc.tile_pool(name="pc_ps", bufs=4, space="PSUM") as pcps,
                ):
                    with nc.named_scope("phaseC"):
                        wq_sb = pc.tile([P, NQC, NDQ * P], BF16)
                        nc.sync.dma_start(wq_sb, wqup_in)
                        prot_sb = pc.tile([P, P], BF16)
                        nc.sync.dma_start(prot_sb, prot_in)
                        latq = pc.tile([P, NQC, NTT, 512], BF16)
                        for tt in range(NTT):
                            for fc in range(NQC):
                                nc.sync.dma_start(latq[:, fc, tt, :],
                                                  ag_q[tt, fc])
                        rqs = []
                        for tt in range(NTT):
                            eng = nc.vector if tt % 2 == 0 else nc.gpsimd
                            acc = pct.tile([P, 512], F32R, name="cacc")
                            nc.scalar.square(acc, latq[:, 0, tt, :])
                            for fc in range(1, NQC):
                                sq = pct.tile([P, 512], F32, name="csq")
                                nc.scalar.square(sq, latq[:, fc, tt, :])
                                eng.tensor_add(acc, acc, sq)
                            st = pcst.tile([1, 512], F32, name="cst")
                            nc.tensor.matmul(st, ones_kf, acc,
                                             start=True, stop=True)
                            srt = pct.tile([1, 512], F32, name="csrt")
                            nc.scalar.activation(srt, st, AF.Ln, bias=eps_q,
                                                 scale=1.0 / (QR * SCALE * SCALE))
                            rk = pct.tile([1, 512], F32R, name="crk")
                            nc.scalar.activation(rk, srt, AF.Exp, scale=-0.5)
                            bc = pcbc.tile([P, 512], F32, name="cbc")
                            nc.tensor.matmul(bc, ones_b, rk,
                                             start=True, stop=True)
                            rq = pc.tile([P, 512], F32, name=f"crq{tt}")
                            nc.vector.tensor_copy(rq, bc)
                            rqs.append(rq)
                        for d in range(NDQ):
                            pss = [pcps.tile([P, 512], F32, name="cqp")
                                   for _ in range(NTT)]
                            for fc in range(NQC):
                                for tt in range(NTT):
                                    nc.tensor.matmul(
                                        pss[tt], wq_sb[:, fc, ds(d * P, P)],
                                        latq[:, fc, tt, :],
                                        start=(fc == 0), stop=(fc == NQC - 1),
                                    )
                            if d < 4:
                                for tt in range(NTT):
                                    nc.vector.tensor_mul(
                                        qT[:, d, ts(tt, 512)], pss[tt],
                                        rqs[tt])
                            else:
                                # roped pair: rotate_half via PE permutation
                                for tt in range(NTT):
                                    qr = pct.tile([P, 512], BF16, name="cqr")
                                    nc.vector.tensor_copy(qr, pss[tt])
                                    rt = pcbc.tile([P, 512], F32, name="cbc")
                                    nc.tensor.matmul(rt, prot_sb, qr,
                                                     start=True, stop=True)
                                    t1 = pct.tile([P, 512], F32, name="ct1")
                                    nc.vector.tensor_mul(
                                        t1, qr, cos_sb[:, ts(tt, 512)])
                                    t2 = pct.tile([P, 512], F32, name="ct2")
                                    nc.vector.tensor_mul(
                                        t2, rt, sin_sb[:, ts(tt, 512)])
                                    nc.vector.tensor_add(t1, t1, t2)
                                    nc.vector.tensor_mul(
                                        qT[:, d, ts(tt, 512)], t1, rqs[tt])

                if DEBUG:
                    nc.sync.dma_start(dbg_qt, qT)
                # ---------------- Phase D: attention ---------------------
                with tc.tile_pool(name="po", bufs=1) as po:
                    o_sb = po.tile([P, 4, S], BF16)
                    with (
                        tc.tile_pool(name="pd_e", bufs=6) as pde,
                        tc.tile_pool(name="pd_a", bufs=8) as pda,
                        tc.tile_pool(name="pd_t", bufs=4) as pdt,
                        tc.tile_pool(name="pd_sc", bufs=3, space="PSUM") as pdsc,
                        tc.tile_pool(name="pd_o", bufs=4, space="PSUM") as pdo,
                        tc.tile_pool(name="pd_den", bufs=1, space="PSUM") as pdd,
                    ):
                        with nc.named_scope("phaseD"):
                            for h in range(4):
                                krop = kre_sb if h % 2 == 0 else kro_sb
                                acc_eng = nc.vector if h % 2 == 0 else nc.gpsimd
                                qp = qT[:, 4 + h // 2, :]
                                ps_o = [pdo.tile([P, 512], F32, name="pso")
                                        for i in range(4)]
                                eaccs = [pda.tile([P, 512], F32R, name="eacc")
                                         for i in range(4)]
                                for jc in range(NTC):
                                    imin = jc // 4
                                    ets = {}
                                    for i in range(imin, 4):
                                        qoff = (jc % 4) * P if i == imin else 0
                                        w = 512 - qoff
                                        ps_sc = pdsc.tile([P, 512], F32,
                                                          name="psc")
                                        nc.tensor.matmul(
                                            ps_sc[:, :w],
                                            kt_sb[:, h, ds(jc * P, P)],
                                            qT[:, h, ds(i * 512 + qoff, w)],
                                            start=True, stop=False)
                                        nc.tensor.matmul(
                                            ps_sc[:, :w],
                                            krop[:, ds(jc * P, P)],
                                            qp[:, ds(i * 512 + qoff, w)],
                                            start=False, stop=True)
                                        et = pde.tile([P, 512], BF16,
                                                      name="et")
                                        nc.scalar.activation(et[:, :w],
                                                             ps_sc[:, :w],
                                                             AF.Exp)
                                        if i == imin:
                                            # beyond col 128 the narrowed
                                            # diag block is fully valid
                                            nc.vector.tensor_mul(
                                                et[:, :P], et[:, :P],
                                                mask0[:, :P])
                                        ets[i] = (et, qoff, w)
                                    for i in range(imin, 4):
                                        et, qoff, w = ets[i]
                                        nc.tensor.matmul(
                                            ps_o[i][:, ds(qoff, w)],
                                            v_sb[:, jc, ds(h * P, P)],
                                            et[:, :w],
                                            start=(jc == 0),
                                            stop=(jc == 4 * i + 3))
                                        if jc == 0:
                                            acc_eng.tensor_copy(eaccs[i], et)
                                        else:
                                            acc_eng.tensor_add(
                                                eaccs[i][:, ds(qoff, w)],
                                                eaccs[i][:, ds(qoff, w)],
                                                et[:, :w])
                                        if jc == 4 * i + 3:
                                            den_ps = pdd.tile([1, 512], F32,
                                                              name="dden")
                                            nc.tensor.matmul(den_ps, ones_kf,
                                                             eaccs[i],
                                                             start=True,
                                                             stop=True)
                                            lnt = pdt.tile([1, 512], F32,
                                                           name="dln")
                                            nc.scalar.activation(lnt, den_ps,
                                                                 AF.Ln)
                                            rk = pdt.tile([1, 512], F32R,
                                                          name="drk")
                                            nc.scalar.activation(rk, lnt,
                                                                 AF.Exp,
                                                                 scale=-1.0)
                                            bc = pdsc.tile([P, 512], F32,
                                                           name="psc")
                                            nc.tensor.matmul(bc, ones_b, rk,
                                                             start=True,
                                                             stop=True)
                                            bcs = pdt.tile([P, 512], F32,
                                                           name="dbcs")
                                            nc.vector.tensor_copy(bcs, bc)
                                            nc.vector.tensor_mul(
                                                o_sb[:, h, ts(i, 512)],
                                                ps_o[i], bcs)

                    if DEBUG:
                        nc.sync.dma_start(dbg_o, o_sb)
                    # ---------------- Phase F: o_proj partial -------------
                    with (
                        tc.tile_pool(name="pf", bufs=1) as pf,
                        tc.tile_pool(name="pf_r", bufs=2) as pfr,
                        tc.tile_pool(name="pf_ps", bufs=4, space="PSUM") as pfp,
                    ):
                        with nc.named_scope("phaseF"):
                            wo_sb = pf.tile([P, 4, HID], BF16)
                            nc.sync.dma_start(wo_sb, wo_in)
                            for tch in range(NTC):
                                orow = pfr.tile([P, HID], BF16, name="orow")
                                pss = [pfp.tile([P, 512], F32, name="fps")
                                       for _ in range(4)]
                                for hh in range(4):
                                    for ct in range(4):
                                        nc.tensor.matmul(
                                            pss[ct],
                                            o_sb[:, hh, ds(tch * P, P)],
                                            wo_sb[:, hh, ts(ct, 512)],
                                            start=(hh == 0), stop=(hh == 3),
                                        )
                                for ct in range(4):
                                    nc.vector.tensor_copy(
                                        orow[:, ts(ct, 512)], pss[ct])
                                nc.sync.dma_start(out_d[ds(tch * P, P), :],
                                                  orow)


_NC_CACHE = None


def _build_nc():
    global _NC_CACHE
    if _NC_CACHE is None:
        nc = bacc.Bacc("TRN2", target_bir_lowering=False, debug=False,
                       num_devices=8)
        with tile.TileContext(nc) as tc:
            _emit(tc)
        nc.compile()
        _NC_CACHE = nc
    return _NC_CACHE


def _shard_inputs(hidden_states, cos, sin, Wq_down, q_gamma, Wq_up,
                  Wkv_down, kv_gamma, Wkv_up, Wo):
    f32 = np.float32
    hid = np.asarray(hidden_states, dtype=f32)
    cos = np.asarray(cos, dtype=f32)
    sin = np.asarray(sin, dtype=f32)
    Wqd = np.asarray(Wq_down, dtype=f32)
    Wkd = np.asarray(Wkv_down, dtype=f32)
    qg = np.asarray(q_gamma, dtype=f32)
    kvg = np.asarray(kv_gamma, dtype=f32)
    Wqu = np.asarray(Wq_up, dtype=f32) * qg[None, :]
    Wku = np.asarray(Wkv_up, dtype=f32) * kvg[None, :]
    Wo = np.asarray(Wo, dtype=f32)

    # combined down-proj weight, kv-first: [ckv | kr | kr | rot | rot | q]
    WckvT = Wkd[:KVR].T                            # [HID, KVR]
    krope = Wkd[KVR:].T                            # [HID, 64]
    krot = np.concatenate([-krope[:, 32:], krope[:, :32]], 1)
    WqdT = Wqd.T                                   # [HID, QR]
    WdT = np.concatenate([WckvT, krope, krope, krot, krot, WqdT], 1)
    wd = np.ascontiguousarray(
        WdT.reshape(NHC, P, NFC, P).transpose(2, 1, 0, 3)).astype(NPBF)

    # rotate_half permutation for the q-rope head pairs
    prot = np.zeros((P, P), dtype=f32)
    for base in (0, 64):
        for t in range(32):
            prot[base + 32 + t, base + t] = -1.0
            prot[base + t, base + 32 + t] = 1.0
    prot = prot.astype(NPBF)

    per_batch = []
    for b in range(B):
        h_sw = np.ascontiguousarray(
            hid[b].T.reshape(NHC, P, S).transpose(1, 0, 2))  # [128, 16, S]
        cT = cos[b].T                               # [64, S]
        sT = sin[b].T
        cos2 = np.ascontiguousarray(np.concatenate([cT, cT], 0))
        sin2 = np.ascontiguousarray(np.concatenate([sT, sT], 0))
        per_batch.append((h_sw, cos2, sin2))

    per_group = []
    for g in range(4):
        bn, br = [], []
        for hl in range(4):
            h = 4 * g + hl
            blk = Wqu[h * QKD:(h + 1) * QKD]       # [192, QR]
            bn.append(blk[:NOPE])
            br.append(blk[NOPE:])
        cols = bn + [np.concatenate([br[0], br[1]], 0),
                     np.concatenate([br[2], br[3]], 0)]
        WquT = np.concatenate(cols, 0).T           # [QR, 768]
        wqup = np.ascontiguousarray(
            WquT.reshape(NQC, P, NDQ * P).transpose(1, 0, 2)).astype(NPBF)
        kb, vb = [], []
        for hl in range(4):
            h = 4 * g + hl
            blk = Wku[h * (NOPE + VD):(h + 1) * (NOPE + VD)]
            kb.append(blk[:NOPE])
            vb.append(blk[NOPE:])
        WkuT = np.concatenate(kb, 0).T             # [KVR, 512]
        WvuT = np.concatenate(vb, 0).T
        wkup = np.ascontiguousarray(
            WkuT.reshape(NKC, P, 512).transpose(1, 0, 2)).astype(NPBF)
        wvup = np.ascontiguousarray(
            WvuT.reshape(NKC, P, 512).transpose(1, 0, 2)).astype(NPBF)
        WoT = Wo[:, g * 512:(g + 1) * 512].T       # [512, HID]
        wo = np.ascontiguousarray(
            WoT.reshape(4, P, HID).transpose(1, 0, 2)).astype(NPBF)
        per_group.append((wqup, wkup, wvup, wo))

    in_maps = []
    for c in range(8):
        b, g = c // 4, c % 4
        h_sw, cos2, sin2 = per_batch[b]
        wqup, wkup, wvup, wo = per_group[g]
        in_maps.append({
            "hid": np.ascontiguousarray(
                h_sw[:, :, g * 512:(g + 1) * 512]).astype(NPBF),
            "cos2": cos2, "sin2": sin2, "wd": wd, "prot": prot,
            "wqup": wqup, "wkup": wkup, "wvup": wvup, "wo": wo,
        })
    return in_maps


def kernel(hidden_states, cos, sin, Wq_down, q_gamma, Wq_up,
           Wkv_down, kv_gamma, Wkv_up, Wo, _trace=False):
    nc = _build_nc()
    in_maps = _shard_inputs(hidden_states, cos, sin, Wq_down, q_gamma, Wq_up,
                            Wkv_down, kv_gamma, Wkv_up, Wo)
    res = run_bass_kernel_spmd(nc, in_maps, core_ids=list(range(8)),
                               trace=_trace)
    out = np.zeros((B, S, HID), dtype=np.float32)
    for c in range(8):
        out[c // 4] += np.asarray(res.results[c]["out"], dtype=np.float32)
    if _trace:
        kernel.last_results = res
    return out
